# revision 30
# baseline (speedup 1.0000x reference)
"""Causal attention (QKV proj + softmax(QK^T/sqrt(d))V) on 8 TRN2 NeuronCores.

Sharding: data-parallel over batch (B=8, one batch element per core).
Per-core kernel, projection/PV matmuls in float32r, S^T matmul in bf16:
  phase 0: x [T,D] -> x^T stored t-block-major [P, tb, dc, 128] via PE
           transposes (SBUF resident); x loaded in 2KB/partition half-row
           DMAs round-robin over 3 rings so the PE never starves
  phase 1: Q^T and K^T evicted as bf16, both fully SBUF-resident (no DRAM
           roundtrip); then V = x @ Wv evicted IN PLACE over the x^T slab
           of the same t-block (x^T morphs into resident V, no roundtrip)
  phase 2: per 512-wide query supertile: S^T = K Q^T in bf16 with causal
           subrange (diagonal key blocks only compute the valid query
           suffix), exp on ACT with fused 1/sqrt(D) scale, a 128x128
           triangular mask on the diagonal sub-block, row sums accumulated
           on the vector engine (one add per key block) finished by 4 tiny
           ones-matmuls per supertile, P@V in f32r, reciprocal normalize,
           store.
"""

import numpy as np

T = 2048
D = 1024
E = 1024
N_CORES = 8
P = 128
TS = 512  # t-slice / supertile width
SCALE = 1.0 / 32.0  # 1/sqrt(D)

DC = D // P  # 8 d-chunks
EC = E // P  # 8 e-chunks
TB = T // P  # 16 t-blocks of 128
NTS = T // TS  # 4 t-slices of 512
JB = TS // P  # 4 q-blocks per supertile
QB = TB // 4  # pt part size in k-blocks


def _attention_kernel(ctx, tc, out, x, wq, wk, wv):
    import concourse.bass as bass
    from concourse import mybir
    from concourse.bass import ts
    from concourse.masks import make_identity

    nc = tc.nc
    f32 = mybir.dt.float32
    f32r = mybir.dt.float32r
    bf16 = mybir.dt.bfloat16
    AF = mybir.ActivationFunctionType

    rings = (nc.sync, nc.gpsimd, nc.scalar)

    # ---- left-side SBUF pools ----
    const = ctx.enter_context(tc.tile_pool(name="const", bufs=1))
    # Sacrificial first DMA per load ring: the first completion semaphore
    # on a ring posts ~3us after the data lands (one-time ring-init cost);
    # eat it on a don't-care 4-byte transfer so the first real x loads
    # post promptly and the PE can start ~2us earlier.
    dummy = const.tile([P, 2], f32)
    nc.sync.dma_start(dummy[:, 0:1], x[0:P, 0:1])
    nc.gpsimd.dma_start(dummy[:, 1:2], x[0:P, 1:2])
    identity_f32 = const.tile([P, P], f32)
    make_identity(nc, identity_f32[:])
    identity = const.tile([P, P], f32r)
    nc.vector.tensor_copy(identity[:], identity_f32[:])
    ones_f32 = const.tile([P, 2], f32)
    nc.vector.memset(ones_f32[:], 1.0)
    ones_col = const.tile([P, 2], f32r)
    nc.vector.tensor_copy(ones_col[:], ones_f32[:])
    # warm the ACT exp table set at program start (off the critical path)
    exp_warm = const.tile([P, 2], f32)
    nc.scalar.activation(exp_warm[:], ones_f32[:], AF.Exp)
    # triangular causal mask tiles (built later, off the hot gpsimd queue)
    tri_f32 = const.tile([P, P], f32)
    tri = const.tile([P, P], f32r)

    kt_pool = ctx.enter_context(tc.tile_pool(name="ktres", bufs=1))
    KT = kt_pool.tile([P, EC, T], bf16)  # K^T[e, t], e = ec*128 + ep
    qt_pool = ctx.enter_context(tc.tile_pool(name="qtres", bufs=1))
    QT = qt_pool.tile([P, EC, T], bf16)  # Q^T[e, t], fully resident

    # ---- right-side work pools ----
    tc.swap_default_side()
    xv_pool = ctx.enter_context(tc.tile_pool(name="xv", bufs=1))
    # x^T t-block-major; after phase 1b each slab is overwritten in place
    # with V[tb] so this same tile is the resident V in phase 2.
    xv = xv_pool.tile([P, TB, DC, P], f32r)  # [dp, tb, dc, tl]
    Vres = xv[:].rearrange("p tb dc e -> p tb (dc e)")  # V[t, e] view
    xa_pool = tc.alloc_tile_pool(name="xa", bufs=12)
    wqk_pool = tc.alloc_tile_pool(name="wqk", bufs=4)
    wvh_pool = tc.alloc_tile_pool(name="wvh", bufs=2)
    tc.swap_default_side()

    # ---- PSUM pools for phases 0/1 ----
    ps_tp = tc.alloc_tile_pool(name="ps_tp", bufs=3, space="PSUM")
    ps_proj = tc.alloc_tile_pool(name="ps_proj", bufs=5, space="PSUM")

    # Prefetch the first two Wq slices on the scalar ring (idle until the
    # first transpose evicts ~12us in; these fresh-tile DMAs have no WAR
    # waits so they cannot head-block the evicts behind them). The rest of
    # the W loads queue behind the x halves on sync/gpsimd, which trickle
    # at transpose pace — too late for eb0/eb1.
    wq_view = wq.bitcast(f32r).rearrange("(dc dp) e -> dp dc e", dp=P)
    wk_view = wk.bitcast(f32r).rearrange("(dc dp) e -> dp dc e", dp=P)
    wr_pre = []
    for eb in range(2):
        wr = wqk_pool.tile([P, DC, P], f32r, tag="wqk", name=f"wrq_{eb}")
        nc.scalar.dma_start(wr[:], wq_view[:, :, ts(eb, P)])
        wr_pre.append(wr)

    # ===== phase 0: x -> x^T via PE transposes =====
    # Half-row x loads (2KB/partition) round-robin over the 3 DMA rings;
    # xa bufs=8 keeps 4 t-blocks in flight so the WAR on slot reuse never
    # head-blocks a ring. 4 transposes share one PSUM bank and evict as a
    # single 2KB copy; ps_tp bufs=3 hides the evict latency.
    # Loads use only sync+gpsimd rings: the scalar engine queue must stay
    # free for psum evicts (a DMA whose WAR dep lags would head-block the
    # evicts behind it and stall the PE).
    HD = D // 2
    for tb in range(TB):
        for h in range(2):
            xa = xa_pool.tile([P, HD], f32r, tag="xa")
            rings[(2 * tb + h) % 2].dma_start(
                xa[:], x[ts(tb, P), h * HD : (h + 1) * HD].bitcast(f32r)
            )
            pt = ps_tp.tile([P, 4, P], f32r)
            for d4 in range(4):
                nc.tensor.transpose(
                    pt[:, d4, :], xa[:, ts(d4, P)], identity[:]
                )
            if h == 0:
                nc.vector.tensor_copy(xv[:, tb, 0:4, :], pt[:])
            else:
                nc.scalar.copy(xv[:, tb, 4:8, :], pt[:])

    # triangular causal mask for the diagonal 128x128 sub-block of phase 2:
    # keep (key_p, query_col) where query_col - key_p >= 0. Built here so
    # its gpsimd ops queue behind the phase-0 x loads, not ahead of them.
    nc.gpsimd.memset(tri_f32[:], 1.0)
    nc.gpsimd.affine_select(
        out=tri_f32[:],
        in_=tri_f32[:],
        compare_op=mybir.AluOpType.is_ge,
        fill=0.0,
        base=0,
        pattern=[[1, P]],
        channel_multiplier=-1,
    )
    nc.vector.tensor_copy(tri[:], tri_f32[:])

    # ======== phase 1a: Q^T and K^T, both bf16 SBUF-resident ========
    ring_i = 0
    for w_view, dst in ((wq_view, QT), (wk_view, KT)):
        for eb in range(EC):
            if dst is QT and eb < 2:
                wr = wr_pre[eb]
            else:
                wr = wqk_pool.tile([P, DC, P], f32r, tag="wqk")
                rings[ring_i % 2].dma_start(wr[:], w_view[:, :, ts(eb, P)])
                ring_i += 1
            for tsl in range(NTS):
                pp = ps_proj.tile([P, TS], f32)
                for dc in range(DC):
                    nc.tensor.matmul(
                        pp[:],
                        wr[:, dc, :],
                        xv[:, 4 * tsl : 4 * tsl + 4, dc, :],
                        start=(dc == 0),
                        stop=(dc == DC - 1),
                    )
                # evict as bf16 into the resident transposed projection
                if tsl % 2 == 0:
                    nc.vector.tensor_copy(dst[:, eb, ts(tsl, TS)], pp[:])
                else:
                    nc.scalar.copy(dst[:, eb, ts(tsl, TS)], pp[:])

    # ========== phase 1b: V = x @ Wv, evicted in place over x^T ==========
    # tb-outer with both Wv halves resident: both psums must be computed
    # before the in-place evicts may overwrite this t-block's x^T slab.
    wv_view = wv.bitcast(f32r).rearrange("(dc dp) e -> dp dc e", dp=P)
    wvhs = []
    for eh in range(E // TS):
        wvh = wvh_pool.tile([P, DC, TS], f32r, tag="wvh", name=f"wvh_{eh}")
        rings[eh % 2].dma_start(wvh[:], wv_view[:, :, ts(eh, TS)])
        wvhs.append(wvh)
    for tb in range(TB):
        pps = []
        for eh in range(E // TS):
            pp = ps_proj.tile([P, TS], f32)
            for dc in range(DC):
                nc.tensor.matmul(
                    pp[:],
                    xv[:, tb, dc, :],
                    wvhs[eh][:, dc, :],
                    start=(dc == 0),
                    stop=(dc == DC - 1),
                )
            pps.append(pp)
        # in-place evicts over the x^T slab of this t-block (WAR: both
        # psum groups above have read the slab before these run)
        nc.scalar.copy(Vres[:, tb, ts(0, TS)], pps[0][:])
        nc.vector.tensor_copy(Vres[:, tb, ts(1, TS)], pps[1][:])

    wvh_pool.release()
    wqk_pool.release()
    xa_pool.release()
    ps_proj.release()
    ps_tp.release()

    # ================= phase 2: attention =================
    ps_s = tc.alloc_tile_pool(name="ps_s", bufs=4, space="PSUM")
    ps_o = tc.alloc_tile_pool(name="ps_o", bufs=3, space="PSUM")
    ps_sum = tc.alloc_tile_pool(name="ps_sum", bufs=1, space="PSUM")

    tc.swap_default_side()
    pt_pool = ctx.enter_context(tc.tile_pool(name="pt", bufs=5))
    rsum_pool = ctx.enter_context(tc.tile_pool(name="rsum", bufs=2))
    rs_pool = ctx.enter_context(tc.tile_pool(name="rs", bufs=8))
    ostg = ctx.enter_context(tc.tile_pool(name="ostg", bufs=3))
    tc.swap_default_side()

    for sup in range(NTS):
        nkb = JB * sup + JB  # key blocks 0..nkb-1
        q0 = sup * TS  # first query column of this supertile in QT
        pt_parts = [
            pt_pool.tile([P, QB, TS], f32r, tag="pt", name=f"ptp_{sup}_0")
        ]
        total = rsum_pool.tile([P, TS], f32r, tag="rsum", name=f"tot_{sup}")

        # --- S^T blocks (bf16, causal subrange) + exp + diag mask ---
        for k in range(nkb):
            j = k - JB * sup  # >= 0 on the diagonal supertile band
            lo = max(j, 0) * P  # first valid query column (within 512)
            ssp = ps_s.tile([P, TS], f32)
            for ec in range(EC):
                nc.tensor.matmul(
                    ssp[:, lo:TS],
                    KT[:, ec, ts(k, P)],
                    QT[:, ec, q0 + lo : q0 + TS],
                    start=(ec == 0),
                    stop=(ec == EC - 1),
                )
            if k // QB >= len(pt_parts):
                pt_parts.append(
                    pt_pool.tile(
                        [P, QB, TS], f32r, tag="pt",
                        name=f"ptp_{sup}_{k // QB}",
                    )
                )
            pk = pt_parts[k // QB][:, k % QB, :]
            nc.scalar.activation(pk[:, lo:TS], ssp[:, lo:TS], AF.Exp, scale=SCALE)
            if j >= 0:
                # triangular mask on the diagonal 128x128 sub-block; columns
                # left of `lo` are never read (PV uses only jq >= j slices,
                # row sums only add the [lo:] subrange).
                nc.vector.tensor_mul(pk[:, lo : lo + P], pk[:, lo : lo + P], tri[:])
            # row-sum accumulation over key blocks on the vector engine
            # (f32r out is full fp32 bits on DVE; "low precision" only
            # applies inside the PE)
            if k == 0:
                nc.vector.tensor_copy(total[:], pk[:])
            else:
                with nc.allow_low_precision(reason="f32r is fp32 on DVE"):
                    nc.vector.tensor_add(
                        total[:, lo:TS], total[:, lo:TS], pk[:, lo:TS]
                    )

        # --- P @ V, batched row-sum finish, normalize, store ---
        pos = ps_sum.tile([P, 2 * JB], f32)
        rss = {}
        for eh in range(E // TS):
            for jq in range(JB):
                qb = JB * sup + jq
                nk = qb + 1
                po = ps_o.tile([P, TS], f32)
                for k in range(nk):
                    nc.tensor.matmul(
                        po[:],
                        pt_parts[k // QB][:, k % QB, ts(jq, P)],
                        Vres[:, k, ts(eh, TS)],
                        start=(k == 0),
                        stop=(k == nk - 1),
                    )
                if eh == 0 and jq == 0:
                    # 4 tiny ones-matmuls reduce `total` over key partitions
                    for j2 in range(JB):
                        nc.tensor.matmul(
                            pos[:, 2 * j2 : 2 * j2 + 2],
                            total[:, ts(j2, P)],
                            ones_col[:],
                            start=True,
                            stop=True,
                        )
                    for j2 in range(JB):
                        rs = rs_pool.tile(
                            [P, 1], f32, tag="rs", name=f"rs_{sup}_{j2}"
                        )
                        nc.vector.reciprocal(rs[:], pos[:, 2 * j2 : 2 * j2 + 1])
                        rss[j2] = rs
                ost = ostg.tile([P, TS], f32, tag="ostage")
                if sup == NTS - 1 and eh == E // TS - 1 and jq == JB - 1:
                    # final output block: split the normalize+store across
                    # ACT/DVE and two DMA rings to shorten the drain tail
                    HT = TS // 2
                    nc.scalar.activation(
                        ost[:, 0:HT], po[:, 0:HT], AF.Copy, scale=rss[jq][:]
                    )
                    nc.vector.tensor_scalar_mul(
                        ost[:, HT:TS], po[:, HT:TS], rss[jq][:]
                    )
                    nc.scalar.dma_start(
                        out[ts(qb, P), eh * TS : eh * TS + HT], ost[:, 0:HT]
                    )
                    nc.sync.dma_start(
                        out[ts(qb, P), eh * TS + HT : (eh + 1) * TS],
                        ost[:, HT:TS],
                    )
                else:
                    nc.scalar.activation(
                        ost[:], po[:], AF.Copy, scale=rss[jq][:]
                    )
                    nc.scalar.dma_start(out[ts(qb, P), ts(eh, TS)], ost[:])

    ps_sum.release()
    ps_o.release()
    ps_s.release()


def build_program():
    from contextlib import ExitStack

    import concourse.bacc as bacc
    import concourse.tile as tile
    from concourse import mybir

    nc = bacc.Bacc("TRN2", target_bir_lowering=False, debug=False)
    f32 = mybir.dt.float32
    x = nc.dram_tensor("x", [T, D], f32, kind="ExternalInput").ap()
    wq = nc.dram_tensor("Wq", [D, E], f32, kind="ExternalInput").ap()
    wk = nc.dram_tensor("Wk", [D, E], f32, kind="ExternalInput").ap()
    wv = nc.dram_tensor("Wv", [D, E], f32, kind="ExternalInput").ap()
    out = nc.dram_tensor("out", [T, E], f32, kind="ExternalOutput").ap()

    with tile.TileContext(nc) as tc:
        with ExitStack() as ctx:
            _attention_kernel(ctx, tc, out, x, wq, wk, wv)
    nc.compile()
    return nc


def kernel(x, Wq, Wk, Wv, _trace=False):
    from concourse.bass_utils import run_bass_kernel_spmd

    x = np.ascontiguousarray(np.asarray(x), dtype=np.float32)
    Wq = np.ascontiguousarray(np.asarray(Wq), dtype=np.float32)
    Wk = np.ascontiguousarray(np.asarray(Wk), dtype=np.float32)
    Wv = np.ascontiguousarray(np.asarray(Wv), dtype=np.float32)
    assert x.shape == (N_CORES, T, D), x.shape

    nc = build_program()
    in_maps = [
        {"x": np.ascontiguousarray(x[b]), "Wq": Wq, "Wk": Wk, "Wv": Wv}
        for b in range(N_CORES)
    ]
    last_err = None
    for attempt in range(3):
        try:
            res = run_bass_kernel_spmd(
                nc, in_maps, core_ids=list(range(N_CORES)), trace=_trace
            )
            break
        except Exception as e:  # transient device wedge: retry
            last_err = e
            import time

            time.sleep(5.0 * (attempt + 1))
    else:
        raise last_err
    out = np.stack([res.results[b]["out"] for b in range(N_CORES)], axis=0)
    if _trace:
        kernel.last_results = res
    return out


kernel.last_results = None
